# revision 16
# baseline (speedup 1.0000x reference)
"""Per-sample Gaussian blur (bilateral-filter reference) on 8 Trainium2 cores.

Math: for each sample b, the reference does a depthwise conv with a separable
normalized Gaussian k x k kernel (k in {5..9} from params[b,0], sigma from
params[b,1]), pad=k//2, then for even k a bilinear resize (H+1,W+1)->(H,W).
Both the 1-D conv and the resize are linear maps along one axis, so the whole
per-sample op is  out_c = A @ X_c @ A^T  with a single banded (|i-j|<=4)
384x384 matrix A = (resize) @ Toeplitz(gauss1d) built on the host.

Device kernel (pure data parallel, one sample per core): for each channel,
two tensor-engine passes with A^T as the moving operand:
  pass1: P^T[w,i] = sum_h X[h,w] * AT[h,i]   (lhsT = X chunk,  rhs = AT)
  pass2: O[i,j]   = sum_w P^T[w,i] * AT[w,j] (lhsT = P^T chunk, rhs = AT)
No transposes needed anywhere. In banded mode the three contraction-chunk
matmuls per output tile use narrowed rhs column windows (A^T is banded), with
start=True on the two outer-chunk matmuls to zero-fill and the middle chunk
accumulating last — saving ~55% of PE columns vs dense.
"""

import numpy as np

_H = 384
_C = 64
_NCORES = 8

# precision / strategy config (hardcoded at submission)
IN_16 = True    # ship x and A^T to the device in fp16 (halves input DMA)
OUT_16 = True   # device writes fp16, host upcasts (halves output DMA)
BANDED = True   # banded matmuls (needs 16-bit operands for 1 cyc/row at N<256)

_prog_cache = {}


def _sigmoid32(v):
    v = np.asarray(v, dtype=np.float32)
    return (1.0 / (1.0 + np.exp(-v.astype(np.float64)))).astype(np.float32)


def _gauss1d(k, sigma):
    c = np.arange(k, dtype=np.float64) - k // 2
    g = np.exp(-(c * c) / (2.0 * float(sigma) ** 2))
    return g / g.sum()


def _build_A(k, sigma, H=_H):
    """Combined conv(+resize for even k) operator along one axis (H x H)."""
    pad = k // 2
    Ho = H + 2 * pad - k + 1  # H odd k, H+1 even k
    g = _gauss1d(k, sigma)
    S = np.zeros((Ho, H), dtype=np.float64)
    for i in range(Ho):
        lo = max(0, i - pad)
        hi = min(H, i - pad + k)
        for m in range(lo, hi):
            S[i, m] = g[m - i + pad]
    if Ho == H:
        return S.astype(np.float32)
    # bilinear resize Ho -> H, half-pixel centers, no antialias
    R = np.zeros((H, Ho), dtype=np.float64)
    scale = Ho / H
    for i in range(H):
        src = (i + 0.5) * scale - 0.5
        i0 = int(np.floor(src))
        t = src - i0
        i0c = min(max(i0, 0), Ho - 1)
        i1c = min(max(i0 + 1, 0), Ho - 1)
        R[i, i0c] += 1.0 - t
        R[i, i1c] += t
    return (R @ S).astype(np.float32)


def _build_program():
    """One SPMD Bass program: x (64,384,384) + at (384,384) -> out."""
    if "nc" in _prog_cache:
        return _prog_cache["nc"]

    from contextlib import ExitStack
    import concourse.bacc as bacc
    import concourse.mybir as mybir
    import concourse.tile as tile

    f32 = mybir.dt.float32
    f32r = mybir.dt.float32r
    f16 = mybir.dt.float16

    dt_in = f16 if IN_16 else f32r
    dt_out = f16 if OUT_16 else f32
    banded = BANDED and IN_16

    nc = bacc.Bacc(None, target_bir_lowering=False)
    x_d = nc.declare_dram_parameter("x", [_C, _H, _H], dt_in, isOutput=False)
    at_d = nc.declare_dram_parameter("at", [_H, _H], dt_in, isOutput=False)
    out_d = nc.declare_dram_parameter("out", [_C, _H, _H], dt_out, isOutput=True)

    # Banded matmul plan per output tile: (chunk, col_lo, col_hi, start).
    # PSUM semantics (probed on HW): start=True resets has_written for the
    # whole bank (data intact); a write to an hw=0 element overwrites, to an
    # hw=1 element accumulates. A^T chunk kc only has nonzero columns in
    # [128*kc-4, 128*kc+131], so each matmul covers just its own band
    # (8-aligned), overwriting fresh columns and accumulating on the two
    # 8..16-column overlaps, which the issue order makes well-defined.
    if banded:
        MM_PLAN = [(0, 0, 136, True), (1, 120, 264, False), (2, 248, 384, False)]
    else:
        MM_PLAN = [(0, 0, 384, True), (1, 0, 384, False), (2, 0, 384, False)]

    with tile.TileContext(nc) as tc, ExitStack() as ctx:
        at_pool = ctx.enter_context(tc.tile_pool(name="at", bufs=1))
        x_pool = ctx.enter_context(tc.tile_pool(name="x", bufs=4))
        pt_pool = ctx.enter_context(tc.tile_pool(name="pt", bufs=2))
        o_pool = ctx.enter_context(tc.tile_pool(name="o", bufs=4))
        # pass1 psum: one merged 3-bank tile per channel (single DVE copy out)
        ps1 = ctx.enter_context(tc.tile_pool(name="ps1", bufs=2, space="PSUM"))
        ps2 = ctx.enter_context(tc.tile_pool(name="ps2", bufs=2, space="PSUM"))

        # A^T resident in SBUF: at_t[p, kc, i] = AT[kc*128 + p, i]
        at_t = at_pool.tile([128, 3, _H], dt_in)
        nc.sync.dma_start(at_t[:], at_d[:].rearrange("(kc p) i -> p kc i", p=128))

        for c in range(_C):
            x_t = x_pool.tile([128, 3, _H], dt_in)
            nc.sync.dma_start(
                x_t[:], x_d[c].rearrange("(kk p) w -> p kk w", p=128)
            )
            pt_t = pt_pool.tile([128, 3, _H], dt_in)
            p1 = ps1.tile([128, 3, 512], f32)
            for m in range(3):
                n_mm = len(MM_PLAN)
                for i_mm, (kc, lo, hi, st) in enumerate(MM_PLAN):
                    nc.tensor.matmul(
                        p1[:, m, lo:hi],
                        x_t[:, kc, 128 * m : 128 * (m + 1)],
                        at_t[:, kc, lo:hi],
                        start=st,
                        stop=(i_mm == n_mm - 1),
                        skip_group_check=True,
                    )
            nc.vector.tensor_copy(pt_t[:], p1[:, :, 0:_H])
            o_t = o_pool.tile([128, 3, _H], dt_out)
            for it in range(3):
                p2 = ps2.tile([128, 512], f32)
                n_mm = len(MM_PLAN)
                for i_mm, (kc, lo, hi, st) in enumerate(MM_PLAN):
                    nc.tensor.matmul(
                        p2[:, lo:hi],
                        pt_t[:, kc, 128 * it : 128 * (it + 1)],
                        at_t[:, kc, lo:hi],
                        start=st,
                        stop=(i_mm == n_mm - 1),
                        skip_group_check=True,
                    )
                # ~13 of 192 pass2 copies go to DVE to balance ACT vs DVE load
                if it == 1 and c % 5 == 4:
                    nc.vector.tensor_copy(o_t[:, it, :], p2[:, 0:_H])
                else:
                    nc.scalar.copy(o_t[:, it, :], p2[:, 0:_H])
            nc.gpsimd.dma_start(
                out_d[c].rearrange("(m p) j -> p m j", p=128), o_t[:]
            )

    nc.finalize()
    _prog_cache["nc"] = nc
    return nc


def kernel(x, params, _trace=False):
    from concourse.bass_utils import run_bass_kernel_spmd
    import concourse.mybir as mybir

    x = np.ascontiguousarray(np.asarray(x, dtype=np.float32))
    params = np.asarray(params, dtype=np.float32)
    B = x.shape[0]
    assert x.shape == (_NCORES, _C, _H, _H), x.shape

    k_int = np.trunc(params[:, 0].astype(np.float32))
    k_sel = np.floor(
        np.float32(5.0) + np.float32(5.0) * _sigmoid32(k_int)
    ).astype(np.int32)
    sigma = np.float32(0.5) + np.float32(4.5) * _sigmoid32(params[:, 1])

    np_in = mybir.dt.np(mybir.dt.float16 if IN_16 else mybir.dt.float32)

    nc = _build_program()
    in_maps = []
    for b in range(B):
        A = _build_A(int(k_sel[b]), float(sigma[b]))
        at = np.ascontiguousarray(A.T)
        in_maps.append(
            {"x": x[b].astype(np_in), "at": at.astype(np_in)}
        )

    res = run_bass_kernel_spmd(
        nc, in_maps, list(range(_NCORES)), trace=_trace
    )
    out = np.stack(
        [np.asarray(res.results[b]["out"]).astype(np.float32) for b in range(B)]
    )
    if _trace:
        return out, res
    return out
